# revision 38
# baseline (speedup 1.0000x reference)
"""CrossCondGPT2 forward on 8 trn2 NeuronCores.

Sharding: 4-way data parallel over batch B=4 x 2-way split of T=1024 within
each batch element (cores 2b, 2b+1 own rows [0,512) and [512,1024) of batch b).

v3: fp8 qkv (DoubleRow) + attention restructure.
  - wq/wk/wv and the LN1 output aT in fp8e4 (x32 / x16 scaling to stay out of
    the denormal range; descale folded into the PSUM->SBUF copies). q/k/v
    matmuls run DoubleRow over feature-chunk pairs (half the PE cycles).
  - pair exchange ships fp8 aT (half of v2's bytes); ReduceScatter(add) writes
    a bf16 sum so peer = sum - own costs no extra quantization.
  - masked attention work skipped: odd key chunks compute/exp/mask only query
    blocks 1,3; AV accumulates those blocks only; no memsets of zero blocks.
  - instruction merges: odd-kt exp pairs + mask mults via strided 3D APs
    (triu2 = [triu|triu]); LN transposes batched 4->1 copy per feature chunk.
  - softmax normalize per head pair: one PE broadcast (sel2 matmul) per pair,
    dual-PSUM DVE mults, no staging copy.
  - attention software-pipelined: scores of pair fo+2 are emitted before AV of
    pair fo so the PE never waits on Act's exp.

kernel(**inputs) takes FULL unsharded inputs, returns [B,T,C] fp32.
"""

import sys

if "/opt/trn_rl_repo" not in sys.path:
    sys.path.insert(0, "/opt/trn_rl_repo")

import numpy as np

import concourse.bacc as bacc
import concourse.mybir as mybir
import concourse.tile as tile

L, B, T, C, NH, HD, HID = 4, 4, 1024, 768, 12, 64, 3072
R = 512  # rows per core
P = 128
NCORES = 8
PAIRS = [[0, 1], [2, 3], [4, 5], [6, 7]]
F32, F32R, BF16 = mybir.dt.float32, mybir.dt.float32r, mybir.dt.bfloat16
F8 = mybir.dt.float8e4
DR = mybir.MatmulPerfMode.DoubleRow
AF = mybir.ActivationFunctionType
ALU = mybir.AluOpType
EPS = 1e-5

CT = C // P  # 6 feature chunks of 128
RT = R // P  # 4 own row tiles
KT = T // P  # 8 key chunks
HT = HID // P  # 24 hidden chunks
PRE_H = 12  # heads whose own-key scores are computed during collective flight

SA = 16.0  # fp8 scale on LN1 output a
SW = 32.0  # fp8 scale on wq/wk/wv
DSC_K = 1.0 / (SA * SW)  # descale for k/v
DSC_Q = DSC_K * 0.125  # descale for q (attn 1/sqrt(hd) folded here)


def build(zq, zp, z2, reps=1):
    """zq/zp/z2: skip qkv / proj / fc2 bias paths when those biases are zero.

    reps>1 repeats the whole forward (h reloaded from x each rep) inside one
    program - used by test.py to measure marginal per-forward device time.
    """
    nc = bacc.Bacc(None, target_bir_lowering=False, debug=False)

    x_in = nc.declare_dram_parameter("x", [R, C], F32, isOutput=False)
    wq_in = nc.declare_dram_parameter("wq", [L, C, C], F8, isOutput=False)
    wk_in = nc.declare_dram_parameter("wk", [L, C, C], F8, isOutput=False)
    wv_in = nc.declare_dram_parameter("wv", [L, C, C], F8, isOutput=False)
    wp_in = nc.declare_dram_parameter("wp", [L, C, C], BF16, isOutput=False)
    w1_in = nc.declare_dram_parameter("w1", [L, C, HID], BF16, isOutput=False)
    w2_in = nc.declare_dram_parameter("w2", [L, HID, C], BF16, isOutput=False)
    bq_in = nc.declare_dram_parameter("bq", [L, C], F32, isOutput=False)
    bk_in = nc.declare_dram_parameter("bk", [L, C], F32, isOutput=False)
    bv_in = nc.declare_dram_parameter("bv", [L, C], BF16, isOutput=False)
    bp_in = nc.declare_dram_parameter("bp", [L, C], F32R, isOutput=False)
    b1_in = nc.declare_dram_parameter("b1", [L, HID], F32, isOutput=False)
    b2_in = nc.declare_dram_parameter("b2", [L, C], F32R, isOutput=False)
    triu2_in = nc.declare_dram_parameter("triu2", [P, 2 * P], BF16, isOutput=False)
    ident_in = nc.declare_dram_parameter("ident", [P, P], BF16, isOutput=False)
    ones_in = nc.declare_dram_parameter("ones_row", [1, P], F32R, isOutput=False)
    onesb_in = nc.declare_dram_parameter("ones_row_b", [1, P], BF16, isOutput=False)
    sel2_in = nc.declare_dram_parameter("sel2", [HD, P], F32R, isOutput=False)
    zeros_in = nc.declare_dram_parameter("zeros64", [HD, R], F32R, isOutput=False)
    out_d = nc.declare_dram_parameter("out", [R, C], F32, isOutput=True)

    from contextlib import ExitStack

    with tile.TileContext(nc) as tc, ExitStack() as _es:
            res = _es.enter_context(tc.tile_pool(name="res", bufs=1))
            wqkv_p = _es.enter_context(tc.tile_pool(name="wqkv", bufs=3))
            wp_p = _es.enter_context(tc.tile_pool(name="wpp", bufs=6))
            w1_p = _es.enter_context(tc.tile_pool(name="w1p", bufs=12))
            w2_p = _es.enter_context(tc.tile_pool(name="w2p", bufs=6))
            at_p = _es.enter_context(tc.tile_pool(name="atp", bufs=2))
            qt_p = _es.enter_context(tc.tile_pool(name="qtp", bufs=6))
            yt_p = _es.enter_context(tc.tile_pool(name="ytp", bufs=6))
            h1_p = _es.enter_context(tc.tile_pool(name="h1p", bufs=24))
            kt_p = _es.enter_context(tc.tile_pool(name="ktp", bufs=6))
            vaug_p = _es.enter_context(tc.tile_pool(name="vaug", bufs=1))
            prexp_p = _es.enter_context(tc.tile_pool(name="prexp", bufs=40))
            rot = _es.enter_context(tc.tile_pool(name="rot", bufs=4))
            mt_p = _es.enter_context(tc.tile_pool(name="mtp", bufs=1))
            attn_p = _es.enter_context(tc.tile_pool(name="attn", bufs=16))
            srs_p = _es.enter_context(tc.tile_pool(name="srs", bufs=2))
            small = _es.enter_context(tc.tile_pool(name="small", bufs=2))
            ps = _es.enter_context(tc.tile_pool(name="ps", bufs=3, space="PSUM"))
            psacc = _es.enter_context(tc.tile_pool(name="psacc", bufs=5, space="PSUM"))
            dram = _es.enter_context(tc.tile_pool(name="dram", bufs=2, space="DRAM"))
            # ---- persistent tiles ----
            ident = res.tile([P, P], BF16, tag="ident")
            nc.sync.dma_start(ident[:], ident_in[:])
            triu2 = res.tile([P, 2, P], BF16, tag="triu2")
            nc.sync.dma_start(triu2[:], triu2_in[:].rearrange("p (b n) -> p b n", n=P))
            ones_row = res.tile([1, P], F32R, tag="ones_row")
            nc.sync.dma_start(ones_row[:], ones_in[:])
            ones_row_b = res.tile([1, P], BF16, tag="ones_row_b")
            nc.sync.dma_start(ones_row_b[:], onesb_in[:])
            sel2 = res.tile([HD, P], F32R, tag="sel2")
            nc.sync.dma_start(sel2[:], sel2_in[:])
            eps_t = res.tile([P, 1], F32, tag="eps")
            nc.vector.memset(eps_t[:], EPS)
            epsq_t = res.tile([P, 1], F32, tag="epsq")
            nc.vector.memset(epsq_t[:], EPS / (SA * SA))
            ones_pp = res.tile([P, NH], BF16, tag="ones_pp")
            nc.vector.memset(ones_pp[:], 1.0)

            for rep in range(reps):
                h = []
                for rt in range(RT):
                    ht_ = res.tile([P, C], F32, tag=f"h{rt}", name=f"h{rt}")
                    q_ = nc.gpsimd if rt % 2 else nc.sync
                    q_.dma_start(ht_[:], x_in[rt * P : (rt + 1) * P, :])
                    h.append(ht_)

                # v_aug tiles persist across layers within a rep; the trailing
                # ones column (softmax denominators ride the AV matmul) is
                # written once here and never overwritten.
                v_aug = [
                    vaug_p.tile([P, NH, HD + 1], BF16, tag=f"va{k}", name=f"va{k}")
                    for k in range(KT)
                ]
                for k in range(KT):
                    nc.vector.tensor_copy(
                        v_aug[k][:, :, HD : HD + 1].rearrange("p h o -> p (h o)"),
                        ones_pp[:],
                    )

                # softmax-recip staging: rows 0 and 32 are overwritten by each
                # pair's reciprocals; every other row must stay zero so the
                # sel2 broadcast matmul contracts them away. One persistent
                # tile per rep, zero-filled once by DMA.
                rr2 = res.tile([HD, R], F32R, tag="rr2p", name="rr2")
                nc.sync.dma_start(rr2[:], zeros_in[:])



                def layernorm(rt, scaled):
                    """Row-major LN of h[rt] -> bf16.

                    scaled=True folds the fp8 SA gain into rstd (a*SA)."""
                    a_t = rot.tile([P, C], BF16, tag="ln_out", bufs=4, name="a_t")
                    stats = small.tile([P, 3, 6], F32, tag="bn_stats", name="stats")
                    xg = h[rt][:].rearrange("p (g d) -> p g d", g=3)
                    for g in range(3):
                        nc.vector.bn_stats(stats[:, g, :], xg[:, g, :])
                    mv = small.tile([P, 2], F32, tag="bn_mv", name="mv")
                    nc.vector.bn_aggr(mv[:], stats[:])
                    std = small.tile([P, 1], F32, tag="bn_std", name="std")
                    if scaled:
                        # std/SA = sqrt(var/SA^2 + eps/SA^2)
                        nc.scalar.activation(
                            std[:], mv[:, 1:2], AF.Sqrt, bias=epsq_t[:],
                            scale=1.0 / (SA * SA),
                        )
                    else:
                        nc.scalar.activation(
                            std[:], mv[:, 1:2], AF.Sqrt, bias=eps_t[:], scale=1.0
                        )
                    rstd = small.tile([P, 1], F32, tag="bn_rstd", name="rstd")
                    nc.vector.reciprocal(rstd[:], std[:])
                    nc.vector.tensor_scalar(
                        out=a_t[:],
                        in0=h[rt][:],
                        scalar1=mv[:, 0:1],
                        scalar2=rstd[:],
                        op0=ALU.subtract,
                        op1=ALU.mult,
                    )
                    return a_t

                def transpose_to_feat(dst, scaled, per_fc_done=None, a_ts=None):
                    """LN all 4 row tiles -> feature-major chunks in dst.

                    dst is a [P, CT, R] tile (fp8 or bf16). Batches the 4
                    transposes of one chunk into a single PSUM tile and one
                    activation copy. per_fc_done(fc) runs right after chunk fc
                    lands (used to stage the pair exchange). a_ts lets callers
                    pass LN outputs that were emitted earlier (interleaved with
                    the previous residual adds)."""
                    if a_ts is None:
                        a_ts = [layernorm(rt, scaled) for rt in range(RT)]
                    for fc in range(CT):
                        tp = ps.tile([P, R], BF16, tag="ps", name="tp")
                        for rt in range(RT):
                            nc.tensor.transpose(
                                tp[:, rt * P : (rt + 1) * P],
                                a_ts[rt][:, fc * P : (fc + 1) * P],
                                ident[:],
                            )
                        nc.scalar.activation(dst[:, fc, :], tp[:], AF.Copy)
                        if per_fc_done is not None:
                            per_fc_done(fc)

                a_ts_pending = None
                for layer in range(L):
                    lsl = slice(layer, layer + 1)

                    # ---- per-layer weight loads (qkv, fp8, DR layout) ----
                    wq = wqkv_p.tile([P, CT, C], F8, tag="wq", bufs=1, name="wq")
                    wk = wqkv_p.tile([P, CT, C], F8, tag="wk", bufs=1, name="wk")
                    wv = wqkv_p.tile([P, CT, C], F8, tag="wv", bufs=1, name="wv")
                    for w_t, w_src in ((wq, wq_in), (wk, wk_in), (wv, wv_in)):
                        nc.sync.dma_start(
                            w_t[:], w_src[layer].rearrange("(k p) m -> p k m", p=P)
                        )

                    if not zq:
                        bq_sb = small.tile([P, CT], F32, tag="bq_sb", name="bq_sb")
                        bk_sb = small.tile([P, CT], F32, tag="bk_sb", name="bk_sb")
                        bv_row = small.tile([1, C], BF16, tag="bv_row", name="bv_row")
                        nc.sync.dma_start(
                            bq_sb[:], bq_in[lsl, :].rearrange("o (f p) -> p (o f)", p=P)
                        )
                        nc.sync.dma_start(
                            bk_sb[:], bk_in[lsl, :].rearrange("o (f p) -> p (o f)", p=P)
                        )
                        nc.sync.dma_start(bv_row[:], bv_in[lsl, :])

                    # ---- LN1 + transpose own rows; stage exchange per chunk ----
                    # AllGather both ranks' fp8 aT; peer = out0 + out1 - own is
                    # EXACT (own cancels in f32), rank-agnostic, no extra
                    # quantization, and stages each chunk only once.
                    rs_in = dram.tile([CT, P, R], F8, tag="rs_in", name="rs_in")
                    rs_out = dram.tile([2, CT, P, R], F8, tag="rs_out", name="rs_out")

                    aT = at_p.tile([P, CT, R], F8, tag="aT", bufs=1, name="aT")

                    def stage(fc):
                        # SP-queue staging keeps the Pool queue free so the
                        # collective kicks as soon as the last chunk lands.
                        nc.sync.dma_start(rs_in[fc], aT[:, fc, :])

                    transpose_to_feat(aT, True, per_fc_done=stage, a_ts=a_ts_pending)
                    a_ts_pending = None
                    nc.gpsimd.collective_compute(
                        "AllGather",
                        ALU.bypass,
                        replica_groups=PAIRS,
                        ins=[rs_in[:]],
                        outs=[rs_out[:]],
                    )
                    # peer reconstruct on the Pool queue, emitted first so it
                    # sits right behind the collective there; the overlap work
                    # below is on other queues. One batched DMA beats 12 small
                    # ones serializing on the Pool queue.
                    aTp = at_p.tile([P, CT, R], F8, tag="aTp", bufs=1, name="aTp")
                    srs_all = srs_p.tile(
                        [P, 2, CT, R], F8, tag="srs", bufs=1, name="srs_all"
                    )
                    nc.gpsimd.dma_start(
                        srs_all[:], rs_out[:].rearrange("g c p r -> p g c r")
                    )
                    for fc in range(CT):
                        ssum = srs_p.tile([P, R], BF16, tag="ssum", bufs=2, name="ssum")
                        nc.gpsimd.tensor_tensor(
                            out=ssum[:], in0=srs_all[:, 0, fc, :],
                            in1=srs_all[:, 1, fc, :], op=ALU.add,
                        )
                        nc.gpsimd.tensor_tensor(
                            out=aTp[:, fc, :], in0=ssum[:], in1=aT[:, fc, :],
                            op=ALU.subtract,
                        )

                    # ======== overlap window (own-data work only) ========
                    def dr_accum(pq, w_t, of0, src):
                        for i2 in range(CT // 2):
                            nc.tensor.matmul(
                                pq[:],
                                w_t[:, 2 * i2 : 2 * i2 + 2, of0 : of0 + P],
                                src[:, 2 * i2 : 2 * i2 + 2, :],
                                start=(i2 == 0),
                                stop=(i2 == CT // 2 - 1),
                                perf_mode=DR,
                            )

                    qT = [qt_p.tile([P, R], BF16, tag="qT", name=f"qT{i}") for i in range(CT)]
                    for of in range(CT):
                        pq = ps.tile([P, R], F32, tag="ps", name="pq")
                        dr_accum(pq, wq, of * P, aT)
                        if zq:
                            # descale+copy on DVE: Act is the bottleneck in
                            # the collective-overlap window
                            nc.vector.tensor_scalar(
                                out=qT[of][:], in0=pq[:], scalar1=DSC_Q,
                                scalar2=None, op0=ALU.mult,
                            )
                        else:
                            nc.scalar.activation(
                                qT[of][:], pq[:], AF.Identity,
                                bias=bq_sb[:, of : of + 1], scale=DSC_Q,
                            )

                    kT = [kt_p.tile([P, T], BF16, tag="kt", name=f"kT{i}") for i in range(CT)]

                    def emit_k(src, r0, on_act=False):
                        # own-half copies go on DVE (Act is the window
                        # bottleneck); peer-half on Act (DVE is the attention
                        # bottleneck)
                        for of in range(CT):
                            pk = ps.tile([P, R], F32, tag="ps", name="pk")
                            dr_accum(pk, wk, of * P, src)
                            dst = kT[of][:, r0 : r0 + R]
                            if not zq:
                                nc.scalar.activation(
                                    dst, pk[:], AF.Identity,
                                    bias=bk_sb[:, of : of + 1], scale=DSC_K,
                                )
                            elif on_act:
                                nc.scalar.activation(dst, pk[:], AF.Copy, scale=DSC_K)
                            else:
                                nc.vector.tensor_scalar(
                                    out=dst, in0=pk[:], scalar1=DSC_K,
                                    scalar2=None, op0=ALU.mult,
                                )

                    emit_k(aT, 0)

                    def emit_v(kt, src, ksl):
                        # v = a[:, ksl] (stationary, DR pairs) @ wv
                        for nh0, nhn in ((0, 8), (8, 4)):
                            n0, nw = nh0 * HD, nhn * HD
                            pv = ps.tile([P, R], F32, tag="ps", name="pv")
                            if not zq:
                                nc.tensor.matmul(
                                    pv[:, :nw],
                                    ones_row_b[:],
                                    bv_row[:, n0 : n0 + nw],
                                    start=True,
                                    stop=False,
                                )
                            for i2 in range(CT // 2):
                                nc.tensor.matmul(
                                    pv[:, :nw],
                                    src[:, 2 * i2 : 2 * i2 + 2, ksl],
                                    wv[:, 2 * i2 : 2 * i2 + 2, n0 : n0 + nw],
                                    start=(zq and i2 == 0),
                                    stop=(i2 == CT // 2 - 1),
                                    perf_mode=DR,
                                )
                            nc.vector.tensor_scalar(
                                out=v_aug[kt][:, nh0 : nh0 + nhn, 0:HD],
                                in0=pv[:, :nw].rearrange("p (h d) -> p h d", d=HD),
                                scalar1=DSC_K,
                                scalar2=None,
                                op0=ALU.mult,
                            )

                    for kt in range(RT):
                        emit_v(kt, aT, slice(kt * P, (kt + 1) * P))

                    # wp prefetch (SP queue; lands during attention)
                    wp = [wp_p.tile([P, C], BF16, tag="wp", name=f"wp{i}") for i in range(CT)]
                    for i in range(CT):
                        nc.sync.dma_start(wp[i][:], wp_in[layer, i * P : (i + 1) * P, :])
                    if not zp:
                        bp_row = small.tile([1, C], F32R, tag="bp_row", name="bp_row")
                        nc.sync.dma_start(bp_row[:], bp_in[lsl, :])

                    def emit_scores(hh, kts, dst):
                        """scoresT+exp+mask for key chunks kts of head hh into
                        dst[kt] ([P, R] bf16). Odd chunks: only query blocks
                        1,3 carry data; blocks 0,2 are never touched (AV skips
                        them)."""
                        fo = hh // 2
                        psl = slice((hh % 2) * HD, (hh % 2) * HD + HD)
                        for kt in kts:
                            et = dst[kt]
                            et3 = et[:].rearrange("p (b n) -> p b n", n=P)
                            if kt % 2 == 0:
                                pscr = ps.tile([P, R], F32, tag="ps", name="pscr")
                                nc.tensor.matmul(
                                    pscr[:],
                                    kT[fo][psl, kt * P : (kt + 1) * P],
                                    qT[fo][psl, :],
                                    start=True,
                                    stop=True,
                                )
                                nc.scalar.activation(et[:], pscr[:], AF.Exp)
                                nc.vector.tensor_tensor(
                                    out=et3[:, slice(0, 3, 2), :],
                                    in0=et3[:, slice(0, 3, 2), :],
                                    in1=triu2[:],
                                    op=ALU.mult,
                                )
                            else:
                                pscr = ps.tile([P, R], F32, tag="ps", name="pscr")
                                ps3 = pscr[:].rearrange("p (b n) -> p b n", n=P)
                                for qs in (1, 3):
                                    sl_ = slice(qs * P, (qs + 1) * P)
                                    nc.tensor.matmul(
                                        pscr[:, sl_],
                                        kT[fo][psl, kt * P : (kt + 1) * P],
                                        qT[fo][psl, sl_],
                                        start=True,
                                        stop=True,
                                    )
                                nc.scalar.activation(
                                    et3[:, slice(1, 4, 2), :],
                                    ps3[:, slice(1, 4, 2), :],
                                    AF.Exp,
                                )
                                nc.vector.tensor_tensor(
                                    out=et3[:, slice(1, 4, 2), :],
                                    in0=et3[:, slice(1, 4, 2), :],
                                    in1=triu2[:],
                                    op=ALU.mult,
                                )

                    # own-key scores for the first PRE_H heads (collective in
                    # flight; uses only qT/kT-own)
                    expT = {}
                    for hh in range(PRE_H):
                        dst = {
                            kt: prexp_p.tile(
                                [P, R], BF16, tag="prexp", name=f"pe{hh}_{kt}"
                            )
                            for kt in range(RT)
                        }
                        emit_scores(hh, range(RT), dst)
                        expT[hh] = dst

                    # ======== peer-data consumers ========
                    emit_k(aTp, R, on_act=True)
                    for kt in range(RT, KT):
                        emit_v(kt, aTp, slice((kt - RT) * P, (kt - RT + 1) * P))

                    def score_rest(hh):
                        """Finish scoring head hh (peer kts; own too if not
                        prescored)."""
                        if hh not in expT:
                            dst = {
                                kt: attn_p.tile(
                                    [P, R], BF16, tag="expT", name=f"et{hh}_{kt}"
                                )
                                for kt in range(RT)
                            }
                            emit_scores(hh, range(RT), dst)
                            expT[hh] = dst
                        for kt in range(RT, KT):
                            expT[hh][kt] = attn_p.tile(
                                [P, R], BF16, tag="expT", name=f"etp{hh}_{kt}"
                            )
                        emit_scores(hh, range(RT, KT), expT[hh])

                    # prologue: fully score the first two pairs
                    for hh in range(min(4, NH)):
                        score_rest(hh)

                    # ---- attention per head pair, pipelined ----
                    yT = [yt_p.tile([P, R], BF16, tag="yT", name=f"yT{i}") for i in range(CT)]
                    for fo in range(CT):
                        h0, h1 = 2 * fo, 2 * fo + 1
                        # AV for the pair
                        pys = []
                        for hh in (h0, h1):
                            py = psacc.tile([P, R], F32, tag="psacc", name="py")
                            ex = expT.pop(hh)
                            for j in range(KT):
                                if j % 2 == 0:
                                    nc.tensor.matmul(
                                        py[: HD + 1, :],
                                        v_aug[j][:, hh, :],
                                        ex[j][:],
                                        start=(j == 0),
                                        stop=False,
                                    )
                                else:
                                    for qs in (1, 3):
                                        sl_ = slice(qs * P, (qs + 1) * P)
                                        nc.tensor.matmul(
                                            py[: HD + 1, sl_],
                                            v_aug[j][:, hh, :],
                                            ex[j][:, sl_],
                                            start=False,
                                            stop=(j == KT - 1 and qs == 3),
                                        )
                            pys.append(py)
                        # keep the PE fed: scores for pair fo+2
                        if 2 * fo + 4 < NH:
                            score_rest(2 * fo + 4)
                            score_rest(2 * fo + 5)
                        # normalize: one broadcast per pair via sel2
                        with nc.allow_low_precision(reason="f32r recip feeds matmul"):
                            nc.vector.reciprocal(rr2[0:1, :], pys[0][HD : HD + 1, :])
                            nc.vector.reciprocal(rr2[32:33, :], pys[1][HD : HD + 1, :])
                        pb = psacc.tile([P, R], F32, tag="psacc", name="pb")
                        nc.tensor.matmul(
                            pb[:], sel2[:], rr2[:], start=True, stop=True
                        )
                        # DVE cannot read two PSUM operands in one op: stage
                        # the broadcast in SBUF once per pair.
                        sb_b = attn_p.tile([P, R], F32, tag="sb_b", bufs=2, name="sb_b")
                        nc.vector.tensor_copy(sb_b[:], pb[:])
                        nc.vector.tensor_tensor(
                            out=yT[fo][0:HD, :],
                            in0=pys[0][:HD, :],
                            in1=sb_b[0:HD, :],
                            op=ALU.mult,
                        )
                        nc.vector.tensor_tensor(
                            out=yT[fo][HD:P, :],
                            in0=pys[1][:HD, :],
                            in1=sb_b[HD:P, :],
                            op=ALU.mult,
                        )

                    # ---- proj + residual (LN2 stats interleaved per row) ----
                    m_as = []
                    for rt in range(RT):
                        for n0, n1 in ((0, 512), (512, 768)):
                            nw = n1 - n0
                            pp = ps.tile([P, R], F32, tag="ps", name="pp")
                            if not zp:
                                nc.tensor.matmul(
                                    pp[:, :nw],
                                    ones_row[:],
                                    bp_row[:, n0:n1],
                                    start=True,
                                    stop=False,
                                )
                            for i in range(CT):
                                nc.tensor.matmul(
                                    pp[:, :nw],
                                    yT[i][:, rt * P : (rt + 1) * P],
                                    wp[i][:, n0:n1],
                                    start=(zp and i == 0),
                                    stop=(i == CT - 1),
                                )
                            nc.vector.tensor_tensor(
                                out=h[rt][:, n0:n1],
                                in0=h[rt][:, n0:n1],
                                in1=pp[:, :nw],
                                op=ALU.add,
                            )
                        m_as.append(layernorm(rt, False))

                    # ---- MLP ----
                    mT = mt_p.tile([P, CT, R], BF16, tag="mT", bufs=1, name="mT")
                    transpose_to_feat(mT, False, a_ts=m_as)

                    b1_sb = small.tile([P, HT], F32, tag="b1_sb", name="b1_sb")
                    nc.sync.dma_start(
                        b1_sb[:], b1_in[lsl, :].rearrange("o (f p) -> p (o f)", p=P)
                    )
                    h1T = [
                        h1_p.tile([P, R], BF16, tag="h1T", name=f"h1T{i}")
                        for i in range(HT)
                    ]
                    for ofg in range(6):
                        w1c = [
                            w1_p.tile([P, 512], BF16, tag="w1c", name=f"w1c{i}")
                            for i in range(CT)
                        ]
                        for i in range(CT):
                            nc.sync.dma_start(
                                w1c[i][:],
                                w1_in[
                                    layer, i * P : (i + 1) * P, ofg * 512 : (ofg + 1) * 512
                                ],
                            )
                        for oi in range(4):
                            of = ofg * 4 + oi
                            pf = ps.tile([P, R], F32, tag="ps", name="pf")
                            for i in range(CT):
                                nc.tensor.matmul(
                                    pf[:],
                                    w1c[i][:, oi * P : (oi + 1) * P],
                                    mT[:, i, :],
                                    start=(i == 0),
                                    stop=(i == CT - 1),
                                )
                            nc.scalar.activation(
                                h1T[of][:], pf[:], AF.Gelu, bias=b1_sb[:, of : of + 1]
                            )

                    # ---- fc2: of-outer, two column passes, h += mlp ----
                    if not z2:
                        b2_row = small.tile([1, C], F32R, tag="b2_row", name="b2_row")
                        nc.sync.dma_start(b2_row[:], b2_in[lsl, :])
                    for n0, n1 in ((0, 512), (512, 768)):
                        nw = n1 - n0
                        pacc = [
                            psacc.tile([P, R], F32, tag="psacc", name=f"pacc{_r}")
                            for _r in range(RT)
                        ]
                        if not z2:
                            for rt in range(RT):
                                nc.tensor.matmul(
                                    pacc[rt][:, :nw],
                                    ones_row[:],
                                    b2_row[:, n0:n1],
                                    start=True,
                                    stop=False,
                                )
                        w2 = [
                            w2_p.tile([P, 512], BF16, tag="w2", name=f"w2_{i}")
                            for i in range(HT)
                        ]
                        for i in range(HT):
                            nc.sync.dma_start(
                                w2[i][:, :nw], w2_in[layer, i * P : (i + 1) * P, n0:n1]
                            )
                            for rt in range(RT):
                                nc.tensor.matmul(
                                    pacc[rt][:, :nw],
                                    h1T[i][:, rt * P : (rt + 1) * P],
                                    w2[i][:, :nw],
                                    start=(z2 and i == 0),
                                    stop=(i == HT - 1),
                                )
                        for rt in range(RT):
                            nc.vector.tensor_tensor(
                                out=h[rt][:, n0:n1],
                                in0=h[rt][:, n0:n1],
                                in1=pacc[rt][:, :nw],
                                op=ALU.add,
                            )
                            if n0 == 512 and layer < L - 1:
                                # next layer's LN1 starts as soon as this row
                                # of the residual stream is final
                                if rt == 0:
                                    a_ts_pending = []
                                a_ts_pending.append(layernorm(rt, True))
                            elif n0 == 512:
                                nc.sync.dma_start(
                                    out_d[rt * P : (rt + 1) * P, n0:], h[rt][:, n0:]
                                )
                            if n0 == 0 and layer == L - 1:
                                nc.sync.dma_start(
                                    out_d[rt * P : (rt + 1) * P, :n1], h[rt][:, :n1]
                                )


    nc.compile()
    return nc


# ------------------------ host side ------------------------

_CACHE = {}


def _prep_inputs(inputs):
    import ml_dtypes

    f32 = np.float32
    bf = ml_dtypes.bfloat16
    f8 = ml_dtypes.float8_e4m3
    g1 = inputs["ln1_g"].astype(f32)[:, :, None]
    b1g = inputs["ln1_b"].astype(f32)
    g2 = inputs["ln2_g"].astype(f32)[:, :, None]
    b2g = inputs["ln2_b"].astype(f32)

    def fold(Wname, bname, g, b, scale=1.0):
        W = inputs[Wname].astype(f32)
        bias = inputs[bname].astype(f32)
        Weff = (g * W) * scale
        beff = (bias + np.einsum("lc,lcd->ld", b, W)) * scale
        return Weff, beff.astype(f32)

    # fp8 qkv: weights shipped pre-scaled by SW; descale (and the 0.125 attn
    # scale for q) happens in the kernel's PSUM->SBUF copies.
    wq, bq = fold("Wq", "bq", g1, b1g)
    wk, bk = fold("Wk", "bk", g1, b1g)
    wv, bv = fold("Wv", "bv", g1, b1g)
    w1, b1 = fold("W1", "b1", g2, b2g)
    bq = bq * 0.125
    bp = inputs["bp"].astype(f32)
    b2 = inputs["b2"].astype(f32)

    triu = np.triu(np.ones((P, P), f32))
    sel2 = np.zeros((HD, P), f32)
    sel2[0, :HD] = 1.0
    sel2[32, HD:] = 1.0

    common = {
        "wq": (wq * SW).astype(f8),
        "wk": (wk * SW).astype(f8),
        "wv": (wv * SW).astype(f8),
        "wp": inputs["Wp"].astype(f32).astype(bf),
        "w1": w1.astype(bf),
        "w2": inputs["W2"].astype(f32).astype(bf),
        "bq": bq,
        "bk": bk,
        "bv": (bv / DSC_K).astype(bf),
        "bp": bp,
        "b1": b1,
        "b2": b2,
        "triu2": np.concatenate([triu, triu], axis=1).astype(bf),
        "ident": np.eye(P, dtype=f32).astype(bf),
        "ones_row": np.ones((1, P), f32),
        "ones_row_b": np.ones((1, P), f32).astype(bf),
        "sel2": sel2,
        "zeros64": np.zeros((HD, R), f32),
    }
    zq = bool(np.all(bq == 0) and np.all(bk == 0) and np.all(bv == 0))
    zp = bool(np.all(bp == 0))
    z2 = bool(np.all(b2 == 0))
    x = inputs["x"].astype(f32)
    shards = [
        np.ascontiguousarray(x[c // 2, (c % 2) * R : (c % 2 + 1) * R, :])
        for c in range(NCORES)
    ]
    return common, shards, (zq, zp, z2)


def get_nc(flags, reps=1):
    key = (flags, reps)
    if key not in _CACHE:
        _CACHE[key] = build(*flags, reps=reps)
    return _CACHE[key]


def kernel(**inputs):
    from concourse.bass_utils import run_bass_kernel_spmd

    common, shards, flags = _prep_inputs(inputs)
    nc = get_nc(flags)
    in_maps = [dict(common, x=shards[c]) for c in range(NCORES)]
    res = run_bass_kernel_spmd(nc, in_maps, list(range(NCORES)), trace=False)
    out = np.empty((B, T, C), np.float32)
    for c in range(NCORES):
        out[c // 2, (c % 2) * R : (c % 2 + 1) * R, :] = res.results[c]["out"]
    return out


if __name__ == "__main__":
    nc = build(True, True, True)
    print("build+compile OK")


# revision 41
# speedup vs baseline: 1.0780x; 1.0780x over previous
"""CrossCondGPT2 forward on 8 trn2 NeuronCores.

Sharding: 4-way data parallel over batch B=4 x 2-way split of T=1024 within
each batch element (cores 2b, 2b+1 own rows [0,512) and [512,1024) of batch b).

v3: fp8 qkv (DoubleRow) + attention restructure.
  - wq/wk/wv and the LN1 output aT in fp8e4 (x32 / x16 scaling to stay out of
    the denormal range; descale folded into the PSUM->SBUF copies). q/k/v
    matmuls run DoubleRow over feature-chunk pairs (half the PE cycles).
  - pair exchange ships fp8 aT (half of v2's bytes); ReduceScatter(add) writes
    a bf16 sum so peer = sum - own costs no extra quantization.
  - masked attention work skipped: odd key chunks compute/exp/mask only query
    blocks 1,3; AV accumulates those blocks only; no memsets of zero blocks.
  - instruction merges: odd-kt exp pairs + mask mults via strided 3D APs
    (triu2 = [triu|triu]); LN transposes batched 4->1 copy per feature chunk.
  - softmax normalize per head pair: one PE broadcast (sel2 matmul) per pair,
    dual-PSUM DVE mults, no staging copy.
  - attention software-pipelined: scores of pair fo+2 are emitted before AV of
    pair fo so the PE never waits on Act's exp.

kernel(**inputs) takes FULL unsharded inputs, returns [B,T,C] fp32.
"""

import sys

if "/opt/trn_rl_repo" not in sys.path:
    sys.path.insert(0, "/opt/trn_rl_repo")

import numpy as np

import concourse.bacc as bacc
import concourse.mybir as mybir
import concourse.tile as tile

L, B, T, C, NH, HD, HID = 4, 4, 1024, 768, 12, 64, 3072
R = 512  # rows per core
P = 128
NCORES = 8
PAIRS = [[0, 1], [2, 3], [4, 5], [6, 7]]
F32, F32R, BF16 = mybir.dt.float32, mybir.dt.float32r, mybir.dt.bfloat16
F8 = mybir.dt.float8e4
DR = mybir.MatmulPerfMode.DoubleRow
AF = mybir.ActivationFunctionType
ALU = mybir.AluOpType
EPS = 1e-5

CT = C // P  # 6 feature chunks of 128
RT = R // P  # 4 own row tiles
KT = T // P  # 8 key chunks
HT = HID // P  # 24 hidden chunks
PRE_H = 12  # heads whose own-key scores are computed during collective flight

SA = 16.0  # fp8 scale on LN1 output a
SW = 32.0  # fp8 scale on wq/wk/wv
DSC_K = 1.0 / (SA * SW)  # descale for k/v
DSC_Q = DSC_K * 0.125  # descale for q (attn 1/sqrt(hd) folded here)


def build(zq, zp, z2, reps=1):
    """zq/zp/z2: skip qkv / proj / fc2 bias paths when those biases are zero.

    reps>1 repeats the whole forward (h reloaded from x each rep) inside one
    program - used by test.py to measure marginal per-forward device time.
    """
    nc = bacc.Bacc(None, target_bir_lowering=False, debug=False)

    x_in = nc.declare_dram_parameter("x", [R, C], F32, isOutput=False)
    wq_in = nc.declare_dram_parameter("wq", [L, C, C], F8, isOutput=False)
    wk_in = nc.declare_dram_parameter("wk", [L, C, C], F8, isOutput=False)
    wv_in = nc.declare_dram_parameter("wv", [L, C, C], F8, isOutput=False)
    wp_in = nc.declare_dram_parameter("wp", [L, C, C], BF16, isOutput=False)
    w1_in = nc.declare_dram_parameter("w1", [L, C, HID], BF16, isOutput=False)
    w2_in = nc.declare_dram_parameter("w2", [L, HID, C], BF16, isOutput=False)
    bq_in = nc.declare_dram_parameter("bq", [L, C], F32, isOutput=False)
    bk_in = nc.declare_dram_parameter("bk", [L, C], F32, isOutput=False)
    bv_in = nc.declare_dram_parameter("bv", [L, C], BF16, isOutput=False)
    bp_in = nc.declare_dram_parameter("bp", [L, C], F32R, isOutput=False)
    b1_in = nc.declare_dram_parameter("b1", [L, HID], F32, isOutput=False)
    b2_in = nc.declare_dram_parameter("b2", [L, C], F32R, isOutput=False)
    triu2_in = nc.declare_dram_parameter("triu2", [P, 2 * P], BF16, isOutput=False)
    ident_in = nc.declare_dram_parameter("ident", [P, P], BF16, isOutput=False)
    ones_in = nc.declare_dram_parameter("ones_row", [1, P], F32R, isOutput=False)
    onesb_in = nc.declare_dram_parameter("ones_row_b", [1, P], BF16, isOutput=False)
    sel2_in = nc.declare_dram_parameter("sel2", [HD, P], F32R, isOutput=False)
    zeros_in = nc.declare_dram_parameter("zeros64", [HD, R], F32R, isOutput=False)
    out_d = nc.declare_dram_parameter("out", [R, C], F32, isOutput=True)

    from contextlib import ExitStack

    with tile.TileContext(nc) as tc, ExitStack() as _es:
            res = _es.enter_context(tc.tile_pool(name="res", bufs=1))
            wqkv_p = _es.enter_context(tc.tile_pool(name="wqkv", bufs=3))
            wp_p = _es.enter_context(tc.tile_pool(name="wpp", bufs=6))
            w1_p = _es.enter_context(tc.tile_pool(name="w1p", bufs=12))
            w2_p = _es.enter_context(tc.tile_pool(name="w2p", bufs=8))
            at_p = _es.enter_context(tc.tile_pool(name="atp", bufs=2))
            qt_p = _es.enter_context(tc.tile_pool(name="qtp", bufs=6))
            yt_p = _es.enter_context(tc.tile_pool(name="ytp", bufs=6))
            h1_p = _es.enter_context(tc.tile_pool(name="h1p", bufs=24))
            kt_p = _es.enter_context(tc.tile_pool(name="ktp", bufs=6))
            vaug_p = _es.enter_context(tc.tile_pool(name="vaug", bufs=1))
            prexp_p = _es.enter_context(tc.tile_pool(name="prexp", bufs=40))
            rot = _es.enter_context(tc.tile_pool(name="rot", bufs=4))
            mt_p = _es.enter_context(tc.tile_pool(name="mtp", bufs=1))
            attn_p = _es.enter_context(tc.tile_pool(name="attn", bufs=16))
            srs_p = _es.enter_context(tc.tile_pool(name="srs", bufs=2))
            small = _es.enter_context(tc.tile_pool(name="small", bufs=2))
            ps = _es.enter_context(tc.tile_pool(name="ps", bufs=3, space="PSUM"))
            psacc = _es.enter_context(tc.tile_pool(name="psacc", bufs=5, space="PSUM"))
            dram = _es.enter_context(tc.tile_pool(name="dram", bufs=2, space="DRAM"))
            # ---- persistent tiles ----
            ident = res.tile([P, P], BF16, tag="ident")
            nc.sync.dma_start(ident[:], ident_in[:])
            triu2 = res.tile([P, 2, P], BF16, tag="triu2")
            nc.sync.dma_start(triu2[:], triu2_in[:].rearrange("p (b n) -> p b n", n=P))
            ones_row = res.tile([1, P], F32R, tag="ones_row")
            nc.sync.dma_start(ones_row[:], ones_in[:])
            ones_row_b = res.tile([1, P], BF16, tag="ones_row_b")
            nc.sync.dma_start(ones_row_b[:], onesb_in[:])
            sel2 = res.tile([HD, P], F32R, tag="sel2")
            nc.sync.dma_start(sel2[:], sel2_in[:])
            eps_t = res.tile([P, 1], F32, tag="eps")
            nc.vector.memset(eps_t[:], EPS)
            epsq_t = res.tile([P, 1], F32, tag="epsq")
            nc.vector.memset(epsq_t[:], EPS / (SA * SA))
            ones_pp = res.tile([P, NH], BF16, tag="ones_pp")
            nc.vector.memset(ones_pp[:], 1.0)

            for rep in range(reps):
                h = []
                for rt in range(RT):
                    ht_ = res.tile([P, C], F32, tag=f"h{rt}", name=f"h{rt}")
                    q_ = nc.gpsimd if rt % 2 else nc.sync
                    q_.dma_start(ht_[:], x_in[rt * P : (rt + 1) * P, :])
                    h.append(ht_)

                # v_aug tiles persist across layers within a rep; the trailing
                # ones column (softmax denominators ride the AV matmul) is
                # written once here and never overwritten.
                v_aug = [
                    vaug_p.tile([P, NH, HD + 1], BF16, tag=f"va{k}", name=f"va{k}")
                    for k in range(KT)
                ]
                for k in range(KT):
                    nc.vector.tensor_copy(
                        v_aug[k][:, :, HD : HD + 1].rearrange("p h o -> p (h o)"),
                        ones_pp[:],
                    )

                # softmax-recip staging: rows 0 and 32 are overwritten by each
                # pair's reciprocals; every other row must stay zero so the
                # sel2 broadcast matmul contracts them away. One persistent
                # tile per rep, zero-filled once by DMA.
                rr2 = res.tile([HD, R], F32R, tag="rr2p", name="rr2")
                nc.sync.dma_start(rr2[:], zeros_in[:])



                def layernorm(rt, scaled):
                    """Row-major LN of h[rt] -> bf16.

                    scaled=True folds the fp8 SA gain into rstd (a*SA)."""
                    a_t = rot.tile([P, C], BF16, tag="ln_out", bufs=4, name="a_t")
                    stats = small.tile([P, 3, 6], F32, tag="bn_stats", name="stats")
                    xg = h[rt][:].rearrange("p (g d) -> p g d", g=3)
                    for g in range(3):
                        nc.vector.bn_stats(stats[:, g, :], xg[:, g, :])
                    mv = small.tile([P, 2], F32, tag="bn_mv", name="mv")
                    nc.vector.bn_aggr(mv[:], stats[:])
                    std = small.tile([P, 1], F32, tag="bn_std", name="std")
                    if scaled:
                        # std/SA = sqrt(var/SA^2 + eps/SA^2)
                        nc.scalar.activation(
                            std[:], mv[:, 1:2], AF.Sqrt, bias=epsq_t[:],
                            scale=1.0 / (SA * SA),
                        )
                    else:
                        nc.scalar.activation(
                            std[:], mv[:, 1:2], AF.Sqrt, bias=eps_t[:], scale=1.0
                        )
                    rstd = small.tile([P, 1], F32, tag="bn_rstd", name="rstd")
                    nc.vector.reciprocal(rstd[:], std[:])
                    # normalize on Act (per-partition scale AP): keeps the DVE
                    # free at the proj->MLP and layer boundaries
                    nb = small.tile([P, 1], F32, tag="bn_nb", name="nb")
                    nc.vector.tensor_scalar(
                        out=nb[:], in0=mv[:, 0:1], scalar1=rstd[:],
                        scalar2=-1.0, op0=ALU.mult, op1=ALU.mult,
                    )
                    nc.scalar.activation(
                        a_t[:], h[rt][:], AF.Identity, bias=nb[:], scale=rstd[:]
                    )
                    return a_t

                def transpose_to_feat(dst, scaled, per_fc_done=None, a_ts=None):
                    """LN all 4 row tiles -> feature-major chunks in dst.

                    dst is a [P, CT, R] tile (fp8 or bf16). Batches the 4
                    transposes of one chunk into a single PSUM tile and one
                    activation copy. per_fc_done(fc) runs right after chunk fc
                    lands (used to stage the pair exchange). a_ts lets callers
                    pass LN outputs that were emitted earlier (interleaved with
                    the previous residual adds)."""
                    if a_ts is None:
                        a_ts = [layernorm(rt, scaled) for rt in range(RT)]
                    for fc in range(CT):
                        tp = ps.tile([P, R], BF16, tag="ps", name="tp")
                        for rt in range(RT):
                            nc.tensor.transpose(
                                tp[:, rt * P : (rt + 1) * P],
                                a_ts[rt][:, fc * P : (fc + 1) * P],
                                ident[:],
                            )
                        nc.scalar.activation(dst[:, fc, :], tp[:], AF.Copy)
                        if per_fc_done is not None:
                            per_fc_done(fc)

                a_ts_pending = None
                for layer in range(L):
                    lsl = slice(layer, layer + 1)

                    # ---- per-layer weight loads (qkv, fp8, DR layout) ----
                    wq = wqkv_p.tile([P, CT, C], F8, tag="wq", bufs=1, name="wq")
                    wk = wqkv_p.tile([P, CT, C], F8, tag="wk", bufs=1, name="wk")
                    wv = wqkv_p.tile([P, CT, C], F8, tag="wv", bufs=1, name="wv")
                    for w_t, w_src in ((wq, wq_in), (wk, wk_in), (wv, wv_in)):
                        nc.sync.dma_start(
                            w_t[:], w_src[layer].rearrange("(k p) m -> p k m", p=P)
                        )

                    if not zq:
                        bq_sb = small.tile([P, CT], F32, tag="bq_sb", name="bq_sb")
                        bk_sb = small.tile([P, CT], F32, tag="bk_sb", name="bk_sb")
                        bv_row = small.tile([1, C], BF16, tag="bv_row", name="bv_row")
                        nc.sync.dma_start(
                            bq_sb[:], bq_in[lsl, :].rearrange("o (f p) -> p (o f)", p=P)
                        )
                        nc.sync.dma_start(
                            bk_sb[:], bk_in[lsl, :].rearrange("o (f p) -> p (o f)", p=P)
                        )
                        nc.sync.dma_start(bv_row[:], bv_in[lsl, :])

                    # ---- LN1 + transpose own rows; stage exchange per chunk ----
                    # AllGather both ranks' fp8 aT; peer = out0 + out1 - own is
                    # EXACT (own cancels in f32), rank-agnostic, no extra
                    # quantization, and stages each chunk only once.
                    rs_in = dram.tile([CT, P, R], F8, tag="rs_in", name="rs_in")
                    rs_out = dram.tile([2, CT, P, R], F8, tag="rs_out", name="rs_out")

                    aT = at_p.tile([P, CT, R], F8, tag="aT", bufs=1, name="aT")

                    def stage(fc):
                        # SP-queue staging keeps the Pool queue free so the
                        # collective kicks as soon as the last chunk lands.
                        nc.sync.dma_start(rs_in[fc], aT[:, fc, :])

                    transpose_to_feat(aT, True, per_fc_done=stage, a_ts=a_ts_pending)
                    a_ts_pending = None
                    nc.gpsimd.collective_compute(
                        "AllGather",
                        ALU.bypass,
                        replica_groups=PAIRS,
                        ins=[rs_in[:]],
                        outs=[rs_out[:]],
                    )
                    # peer reconstruct on the Pool queue, emitted first so it
                    # sits right behind the collective there; the overlap work
                    # below is on other queues. One batched DMA beats 12 small
                    # ones serializing on the Pool queue.
                    aTp = at_p.tile([P, CT, R], F8, tag="aTp", bufs=1, name="aTp")
                    srs_all = srs_p.tile(
                        [P, 2, CT, R], F8, tag="srs", bufs=1, name="srs_all"
                    )
                    nc.gpsimd.dma_start(
                        srs_all[:], rs_out[:].rearrange("g c p r -> p g c r")
                    )
                    for fc in range(CT):
                        ssum = srs_p.tile([P, R], BF16, tag="ssum", bufs=2, name="ssum")
                        nc.gpsimd.tensor_tensor(
                            out=ssum[:], in0=srs_all[:, 0, fc, :],
                            in1=srs_all[:, 1, fc, :], op=ALU.add,
                        )
                        nc.gpsimd.tensor_tensor(
                            out=aTp[:, fc, :], in0=ssum[:], in1=aT[:, fc, :],
                            op=ALU.subtract,
                        )

                    # ======== overlap window (own-data work only) ========
                    def dr_accum(pq, w_t, of0, src):
                        for i2 in range(CT // 2):
                            nc.tensor.matmul(
                                pq[:],
                                w_t[:, 2 * i2 : 2 * i2 + 2, of0 : of0 + P],
                                src[:, 2 * i2 : 2 * i2 + 2, :],
                                start=(i2 == 0),
                                stop=(i2 == CT // 2 - 1),
                                perf_mode=DR,
                            )

                    qT = [qt_p.tile([P, R], BF16, tag="qT", name=f"qT{i}") for i in range(CT)]
                    for of in range(CT):
                        pq = ps.tile([P, R], F32, tag="ps", name="pq")
                        dr_accum(pq, wq, of * P, aT)
                        if zq:
                            # descale+copy on DVE: Act is the bottleneck in
                            # the collective-overlap window
                            nc.vector.tensor_scalar(
                                out=qT[of][:], in0=pq[:], scalar1=DSC_Q,
                                scalar2=None, op0=ALU.mult,
                            )
                        else:
                            nc.scalar.activation(
                                qT[of][:], pq[:], AF.Identity,
                                bias=bq_sb[:, of : of + 1], scale=DSC_Q,
                            )

                    kT = [kt_p.tile([P, T], BF16, tag="kt", name=f"kT{i}") for i in range(CT)]

                    def emit_k(src, r0, on_act=False):
                        # own-half copies go on DVE (Act is the window
                        # bottleneck); peer-half on Act (DVE is the attention
                        # bottleneck)
                        for of in range(CT):
                            pk = ps.tile([P, R], F32, tag="ps", name="pk")
                            dr_accum(pk, wk, of * P, src)
                            dst = kT[of][:, r0 : r0 + R]
                            if not zq:
                                nc.scalar.activation(
                                    dst, pk[:], AF.Identity,
                                    bias=bk_sb[:, of : of + 1], scale=DSC_K,
                                )
                            elif on_act:
                                nc.scalar.activation(dst, pk[:], AF.Copy, scale=DSC_K)
                            else:
                                nc.vector.tensor_scalar(
                                    out=dst, in0=pk[:], scalar1=DSC_K,
                                    scalar2=None, op0=ALU.mult,
                                )

                    emit_k(aT, 0)

                    def emit_v(kt, src, ksl):
                        # v = a[:, ksl] (stationary, DR pairs) @ wv
                        for nh0, nhn in ((0, 8), (8, 4)):
                            n0, nw = nh0 * HD, nhn * HD
                            pv = ps.tile([P, R], F32, tag="ps", name="pv")
                            if not zq:
                                nc.tensor.matmul(
                                    pv[:, :nw],
                                    ones_row_b[:],
                                    bv_row[:, n0 : n0 + nw],
                                    start=True,
                                    stop=False,
                                )
                            for i2 in range(CT // 2):
                                nc.tensor.matmul(
                                    pv[:, :nw],
                                    src[:, 2 * i2 : 2 * i2 + 2, ksl],
                                    wv[:, 2 * i2 : 2 * i2 + 2, n0 : n0 + nw],
                                    start=(zq and i2 == 0),
                                    stop=(i2 == CT // 2 - 1),
                                    perf_mode=DR,
                                )
                            nc.vector.tensor_scalar(
                                out=v_aug[kt][:, nh0 : nh0 + nhn, 0:HD],
                                in0=pv[:, :nw].rearrange("p (h d) -> p h d", d=HD),
                                scalar1=DSC_K,
                                scalar2=None,
                                op0=ALU.mult,
                            )

                    for kt in range(RT):
                        emit_v(kt, aT, slice(kt * P, (kt + 1) * P))

                    # wp prefetch (SP queue; lands during attention)
                    wp = [wp_p.tile([P, C], BF16, tag="wp", name=f"wp{i}") for i in range(CT)]
                    for i in range(CT):
                        nc.sync.dma_start(wp[i][:], wp_in[layer, i * P : (i + 1) * P, :])
                    if not zp:
                        bp_row = small.tile([1, C], F32R, tag="bp_row", name="bp_row")
                        nc.sync.dma_start(bp_row[:], bp_in[lsl, :])

                    def emit_scores(hh, kts, dst):
                        """scoresT+exp+mask for key chunks kts of head hh into
                        dst[kt] ([P, R] bf16). Odd chunks: only query blocks
                        1,3 carry data; blocks 0,2 are never touched (AV skips
                        them)."""
                        fo = hh // 2
                        psl = slice((hh % 2) * HD, (hh % 2) * HD + HD)
                        for kt in kts:
                            et = dst[kt]
                            et3 = et[:].rearrange("p (b n) -> p b n", n=P)
                            if kt % 2 == 0:
                                pscr = ps.tile([P, R], F32, tag="ps", name="pscr")
                                nc.tensor.matmul(
                                    pscr[:],
                                    kT[fo][psl, kt * P : (kt + 1) * P],
                                    qT[fo][psl, :],
                                    start=True,
                                    stop=True,
                                )
                                nc.scalar.activation(et[:], pscr[:], AF.Exp)
                                nc.vector.tensor_tensor(
                                    out=et3[:, slice(0, 3, 2), :],
                                    in0=et3[:, slice(0, 3, 2), :],
                                    in1=triu2[:],
                                    op=ALU.mult,
                                )
                            else:
                                pscr = ps.tile([P, R], F32, tag="ps", name="pscr")
                                ps3 = pscr[:].rearrange("p (b n) -> p b n", n=P)
                                for qs in (1, 3):
                                    sl_ = slice(qs * P, (qs + 1) * P)
                                    nc.tensor.matmul(
                                        pscr[:, sl_],
                                        kT[fo][psl, kt * P : (kt + 1) * P],
                                        qT[fo][psl, sl_],
                                        start=True,
                                        stop=True,
                                    )
                                nc.scalar.activation(
                                    et3[:, slice(1, 4, 2), :],
                                    ps3[:, slice(1, 4, 2), :],
                                    AF.Exp,
                                )
                                nc.vector.tensor_tensor(
                                    out=et3[:, slice(1, 4, 2), :],
                                    in0=et3[:, slice(1, 4, 2), :],
                                    in1=triu2[:],
                                    op=ALU.mult,
                                )

                    # own-key scores for the first PRE_H heads (collective in
                    # flight; uses only qT/kT-own)
                    expT = {}
                    for hh in range(PRE_H):
                        dst = {
                            kt: prexp_p.tile(
                                [P, R], BF16, tag="prexp", name=f"pe{hh}_{kt}"
                            )
                            for kt in range(RT)
                        }
                        emit_scores(hh, range(RT), dst)
                        expT[hh] = dst

                    # ======== peer-data consumers ========
                    emit_k(aTp, R, on_act=True)
                    for kt in range(RT, KT):
                        emit_v(kt, aTp, slice((kt - RT) * P, (kt - RT + 1) * P))

                    def score_rest(hh):
                        """Finish scoring head hh (peer kts; own too if not
                        prescored)."""
                        if hh not in expT:
                            dst = {
                                kt: attn_p.tile(
                                    [P, R], BF16, tag="expT", name=f"et{hh}_{kt}"
                                )
                                for kt in range(RT)
                            }
                            emit_scores(hh, range(RT), dst)
                            expT[hh] = dst
                        for kt in range(RT, KT):
                            expT[hh][kt] = attn_p.tile(
                                [P, R], BF16, tag="expT", name=f"etp{hh}_{kt}"
                            )
                        emit_scores(hh, range(RT, KT), expT[hh])

                    # prologue: fully score the first two pairs
                    for hh in range(min(4, NH)):
                        score_rest(hh)

                    # ---- attention per head pair, pipelined ----
                    yT = [yt_p.tile([P, R], BF16, tag="yT", name=f"yT{i}") for i in range(CT)]
                    for fo in range(CT):
                        h0, h1 = 2 * fo, 2 * fo + 1
                        # AV for the pair
                        pys = []
                        for hh in (h0, h1):
                            py = psacc.tile([P, R], F32, tag="psacc", name="py")
                            ex = expT.pop(hh)
                            for j in range(KT):
                                if j % 2 == 0:
                                    nc.tensor.matmul(
                                        py[: HD + 1, :],
                                        v_aug[j][:, hh, :],
                                        ex[j][:],
                                        start=(j == 0),
                                        stop=False,
                                    )
                                else:
                                    for qs in (1, 3):
                                        sl_ = slice(qs * P, (qs + 1) * P)
                                        nc.tensor.matmul(
                                            py[: HD + 1, sl_],
                                            v_aug[j][:, hh, :],
                                            ex[j][:, sl_],
                                            start=False,
                                            stop=(j == KT - 1 and qs == 3),
                                        )
                            pys.append(py)
                        # keep the PE fed: scores for pair fo+2
                        if 2 * fo + 4 < NH:
                            score_rest(2 * fo + 4)
                            score_rest(2 * fo + 5)
                        # normalize: one broadcast per pair via sel2
                        with nc.allow_low_precision(reason="f32r recip feeds matmul"):
                            nc.vector.reciprocal(rr2[0:1, :], pys[0][HD : HD + 1, :])
                            nc.vector.reciprocal(rr2[32:33, :], pys[1][HD : HD + 1, :])
                        pb = psacc.tile([P, R], F32, tag="psacc", name="pb")
                        nc.tensor.matmul(
                            pb[:], sel2[:], rr2[:], start=True, stop=True
                        )
                        # DVE cannot read two PSUM operands in one op: stage
                        # the broadcast in SBUF once per pair.
                        sb_b = attn_p.tile([P, R], F32, tag="sb_b", bufs=2, name="sb_b")
                        nc.vector.tensor_copy(sb_b[:], pb[:])
                        nc.vector.tensor_tensor(
                            out=yT[fo][0:HD, :],
                            in0=pys[0][:HD, :],
                            in1=sb_b[0:HD, :],
                            op=ALU.mult,
                        )
                        nc.vector.tensor_tensor(
                            out=yT[fo][HD:P, :],
                            in0=pys[1][:HD, :],
                            in1=sb_b[HD:P, :],
                            op=ALU.mult,
                        )

                    # ---- proj + residual (LN2 stats interleaved per row) ----
                    # i-outer over 4 parallel PSUM banks: the i<5 matmuls run
                    # while the last pair's yT is still normalizing.
                    m_as = []
                    for n0, n1 in ((0, 512), (512, 768)):
                        nw = n1 - n0
                        pps = [
                            psacc.tile([P, R], F32, tag="psacc", name=f"pp{_r}")
                            for _r in range(RT)
                        ]
                        if not zp:
                            for rt in range(RT):
                                nc.tensor.matmul(
                                    pps[rt][:, :nw],
                                    ones_row[:],
                                    bp_row[:, n0:n1],
                                    start=True,
                                    stop=False,
                                )
                        for i in range(CT):
                            for rt in range(RT):
                                nc.tensor.matmul(
                                    pps[rt][:, :nw],
                                    yT[i][:, rt * P : (rt + 1) * P],
                                    wp[i][:, n0:n1],
                                    start=(zp and i == 0),
                                    stop=(i == CT - 1),
                                )
                        for rt in range(RT):
                            nc.vector.tensor_tensor(
                                out=h[rt][:, n0:n1],
                                in0=h[rt][:, n0:n1],
                                in1=pps[rt][:, :nw],
                                op=ALU.add,
                            )
                            if n0 == 512:
                                m_as.append(layernorm(rt, False))

                    # ---- MLP ----
                    mT = mt_p.tile([P, CT, R], BF16, tag="mT", bufs=1, name="mT")
                    transpose_to_feat(mT, False, a_ts=m_as)

                    b1_sb = small.tile([P, HT], F32, tag="b1_sb", name="b1_sb")
                    nc.sync.dma_start(
                        b1_sb[:], b1_in[lsl, :].rearrange("o (f p) -> p (o f)", p=P)
                    )
                    h1T = [
                        h1_p.tile([P, R], BF16, tag="h1T", name=f"h1T{i}")
                        for i in range(HT)
                    ]
                    for ofg in range(6):
                        w1c = [
                            w1_p.tile([P, 512], BF16, tag="w1c", name=f"w1c{i}")
                            for i in range(CT)
                        ]
                        for i in range(CT):
                            nc.sync.dma_start(
                                w1c[i][:],
                                w1_in[
                                    layer, i * P : (i + 1) * P, ofg * 512 : (ofg + 1) * 512
                                ],
                            )
                        for oi in range(4):
                            of = ofg * 4 + oi
                            pf = ps.tile([P, R], F32, tag="ps", name="pf")
                            for i in range(CT):
                                nc.tensor.matmul(
                                    pf[:],
                                    w1c[i][:, oi * P : (oi + 1) * P],
                                    mT[:, i, :],
                                    start=(i == 0),
                                    stop=(i == CT - 1),
                                )
                            nc.scalar.activation(
                                h1T[of][:], pf[:], AF.Gelu, bias=b1_sb[:, of : of + 1]
                            )

                    # ---- fc2: of-outer, two column passes, h += mlp ----
                    if not z2:
                        b2_row = small.tile([1, C], F32R, tag="b2_row", name="b2_row")
                        nc.sync.dma_start(b2_row[:], b2_in[lsl, :])
                    for n0, n1 in ((0, 512), (512, 768)):
                        nw = n1 - n0
                        pacc = [
                            psacc.tile([P, R], F32, tag="psacc", name=f"pacc{_r}")
                            for _r in range(RT)
                        ]
                        if not z2:
                            for rt in range(RT):
                                nc.tensor.matmul(
                                    pacc[rt][:, :nw],
                                    ones_row[:],
                                    b2_row[:, n0:n1],
                                    start=True,
                                    stop=False,
                                )
                        w2 = [
                            w2_p.tile([P, 512], BF16, tag="w2", name=f"w2_{i}")
                            for i in range(HT)
                        ]
                        for i in range(HT):
                            nc.sync.dma_start(
                                w2[i][:, :nw], w2_in[layer, i * P : (i + 1) * P, n0:n1]
                            )
                            for rt in range(RT):
                                nc.tensor.matmul(
                                    pacc[rt][:, :nw],
                                    h1T[i][:, rt * P : (rt + 1) * P],
                                    w2[i][:, :nw],
                                    start=(z2 and i == 0),
                                    stop=(i == HT - 1),
                                )
                        for rt in range(RT):
                            nc.vector.tensor_tensor(
                                out=h[rt][:, n0:n1],
                                in0=h[rt][:, n0:n1],
                                in1=pacc[rt][:, :nw],
                                op=ALU.add,
                            )
                            if n0 == 512 and layer < L - 1:
                                # next layer's LN1 starts as soon as this row
                                # of the residual stream is final
                                if rt == 0:
                                    a_ts_pending = []
                                a_ts_pending.append(layernorm(rt, True))
                            elif n0 == 512:
                                nc.sync.dma_start(
                                    out_d[rt * P : (rt + 1) * P, n0:], h[rt][:, n0:]
                                )
                            if n0 == 0 and layer == L - 1:
                                nc.sync.dma_start(
                                    out_d[rt * P : (rt + 1) * P, :n1], h[rt][:, :n1]
                                )


    nc.compile()
    return nc


# ------------------------ host side ------------------------

_CACHE = {}


def _prep_inputs(inputs):
    import ml_dtypes

    f32 = np.float32
    bf = ml_dtypes.bfloat16
    f8 = ml_dtypes.float8_e4m3
    g1 = inputs["ln1_g"].astype(f32)[:, :, None]
    b1g = inputs["ln1_b"].astype(f32)
    g2 = inputs["ln2_g"].astype(f32)[:, :, None]
    b2g = inputs["ln2_b"].astype(f32)

    def fold(Wname, bname, g, b, scale=1.0):
        W = inputs[Wname].astype(f32)
        bias = inputs[bname].astype(f32)
        Weff = (g * W) * scale
        beff = (bias + np.einsum("lc,lcd->ld", b, W)) * scale
        return Weff, beff.astype(f32)

    # fp8 qkv: weights shipped pre-scaled by SW; descale (and the 0.125 attn
    # scale for q) happens in the kernel's PSUM->SBUF copies.
    wq, bq = fold("Wq", "bq", g1, b1g)
    wk, bk = fold("Wk", "bk", g1, b1g)
    wv, bv = fold("Wv", "bv", g1, b1g)
    w1, b1 = fold("W1", "b1", g2, b2g)
    bq = bq * 0.125
    bp = inputs["bp"].astype(f32)
    b2 = inputs["b2"].astype(f32)

    triu = np.triu(np.ones((P, P), f32))
    sel2 = np.zeros((HD, P), f32)
    sel2[0, :HD] = 1.0
    sel2[32, HD:] = 1.0

    common = {
        "wq": (wq * SW).astype(f8),
        "wk": (wk * SW).astype(f8),
        "wv": (wv * SW).astype(f8),
        "wp": inputs["Wp"].astype(f32).astype(bf),
        "w1": w1.astype(bf),
        "w2": inputs["W2"].astype(f32).astype(bf),
        "bq": bq,
        "bk": bk,
        "bv": (bv / DSC_K).astype(bf),
        "bp": bp,
        "b1": b1,
        "b2": b2,
        "triu2": np.concatenate([triu, triu], axis=1).astype(bf),
        "ident": np.eye(P, dtype=f32).astype(bf),
        "ones_row": np.ones((1, P), f32),
        "ones_row_b": np.ones((1, P), f32).astype(bf),
        "sel2": sel2,
        "zeros64": np.zeros((HD, R), f32),
    }
    zq = bool(np.all(bq == 0) and np.all(bk == 0) and np.all(bv == 0))
    zp = bool(np.all(bp == 0))
    z2 = bool(np.all(b2 == 0))
    x = inputs["x"].astype(f32)
    shards = [
        np.ascontiguousarray(x[c // 2, (c % 2) * R : (c % 2 + 1) * R, :])
        for c in range(NCORES)
    ]
    return common, shards, (zq, zp, z2)


def get_nc(flags, reps=1):
    key = (flags, reps)
    if key not in _CACHE:
        _CACHE[key] = build(*flags, reps=reps)
    return _CACHE[key]


def kernel(**inputs):
    from concourse.bass_utils import run_bass_kernel_spmd

    common, shards, flags = _prep_inputs(inputs)
    nc = get_nc(flags)
    in_maps = [dict(common, x=shards[c]) for c in range(NCORES)]
    res = run_bass_kernel_spmd(nc, in_maps, list(range(NCORES)), trace=False)
    out = np.empty((B, T, C), np.float32)
    for c in range(NCORES):
        out[c // 2, (c % 2) * R : (c % 2 + 1) * R, :] = res.results[c]["out"]
    return out


if __name__ == "__main__":
    nc = build(True, True, True)
    print("build+compile OK")
